# revision 23
# baseline (speedup 1.0000x reference)
"""2-layer GAT (GATConv x2, mean over 4 heads) on 8 Trainium2 NeuronCores.

Strategy (dst-segment parallel, per-tile batched dma_gather):
  - Host: self-loops are appended as ordinary edges.  Nodes are split into
    lo/hi halves of 25600 (so gather indices fit int16), each half packed
    by greedy (in-degree+1) bin-packing into 200 tiles of 128 (50 tiles
    per core total).  Each core owns all edges of its 50 tiles, so the
    segment softmax never crosses cores.
  - G rows (fp16, 1280B stride for dma_gather): [xh(512)|a_s(4)|a_d(4)|pad],
    4-way interleaved within 4-tile groups so phase-A stores move 4 rows
    per DMA descriptor.
  - Phase B per tile (KT = 2*K2 chunks = KT*128 edge slots):
      * TWO dma_gathers (InstDMAGatherAnt, single_packet=False) fetch all
        source rows: tile edges are grouped by src half, lo edges in
        chunks [0,K2), hi in [K2,KT); pad slots gather row 0 (masked out
        later via dstl=-1),
      * ONE [128,1]-offset indirect DMA fetches the tile's own a_d vector
        straight out of G via element_offset=516,
      * ONE DVE op builds all KT edge->dst one-hot masks (broadcast APs),
      * per-edge a_d = maskT.T @ ad_tile: masks are PE-transposed in
        batches of 8 per PSUM bank, then KT 4-col matmuls,
      * alpha = a_s + a_d, leaky-relu on DVE, exp on the Scalar engine
        (softmax max-shift skipped: alpha is O(1), softmax shift-invariant),
      * ONE in-place DVE op scales all messages by ex,
      * per chunk 2 PE matmuls: out += mask.T @ msg (512 col) and
        den += mask.T @ ex (4 col), accumulating in separate PSUM banks.
    Head-mean + 1/den normalization is a handful of batched DVE ops.
  - h tiles are PE-transposed, AllGather'd in fp16, and consumed as
    ready-made lhsT slabs by layer-2's phase A.
"""

import numpy as np

P = 128
NCORES = 8

_CACHE = {}


def _pack_half(weight, lo_deg, hi_deg, nodes, tile0, NTH, newid, cap):
    """Greedy largest-first packing of `nodes` into NTH tiles of 128,
    keeping each tile's lo-source and hi-source edge counts under cap."""
    import heapq
    heap = [(0, tile0 + t) for t in range(NTH)]
    heapq.heapify(heap)
    fill = {tile0 + t: 0 for t in range(NTH)}
    lo = {tile0 + t: 0 for t in range(NTH)}
    hi = {tile0 + t: 0 for t in range(NTH)}
    for node in nodes:                       # pre-sorted by weight desc
        popped = []
        placed = None
        while heap:
            cnt, t = heapq.heappop(heap)
            if (lo[t] + lo_deg[node] <= cap
                    and hi[t] + hi_deg[node] <= cap):
                placed = (cnt, t)
                break
            popped.append((cnt, t))
        if placed is None:                   # overflow: least-loaded tile
            placed = popped.pop(0)
        for item in popped:
            heapq.heappush(heap, item)
        cnt, t = placed
        newid[node] = t * P + fill[t]
        fill[t] += 1
        lo[t] += lo_deg[node]
        hi[t] += hi_deg[node]
        if fill[t] < P:
            heapq.heappush(heap, (int(cnt + weight[node]), t))


def _rho(n):
    """node-id -> G-row-id: 2-way interleave within groups of 2 tiles
    (so phase-A stores move 2 rows per descriptor).  Preserves 256-blocks,
    so lo/hi halves and per-core blocks map to themselves."""
    return ((n >> 8) << 8) | ((n & 127) << 1) | ((n >> 7) & 1)


def _host_prep(x, edge_index, W1, att_src1, att_dst1, b1, W2, att_src2,
               att_dst2, b2):
    bf = np.float16

    N, IN_F = x.shape
    HEADS, HID = att_src1.shape
    OUT_F = att_src2.shape[1]
    TPC = -(-int(N * 1.02) // (NCORES * P))   # ~2% slack for packing
    NT = NCORES * TPC
    NP_ = NT * P
    NPH = NP_ // 2
    assert NP_ >= N and NT % 2 == 0

    src0 = np.asarray(edge_index[0], np.int64)
    dst0 = np.asarray(edge_index[1], np.int64)
    w = np.zeros(NP_, np.int64)
    w[:N] = np.bincount(dst0, minlength=N)              # self loop separate

    # split nodes into lo/hi halves (snake over weight-desc order), pack each
    order = np.argsort(-w, kind="stable")
    half = np.empty(NP_, np.int8)
    half[order[0::2]] = 0
    half[order[1::2]] = 1
    lo_deg = np.zeros(NP_, np.int64)
    np.add.at(lo_deg, dst0, (half[src0] == 0))
    hi_deg = w - lo_deg
    ssum = max(lo_deg[half == 0].sum(), hi_deg[half == 0].sum(),
               lo_deg[half == 1].sum(), hi_deg[half == 1].sum())
    cap = int(np.ceil(ssum / (NT // 2) / P)) * P
    newid = np.empty(NP_, np.int64)
    _pack_half(w, lo_deg, hi_deg, order[0::2], 0, NT // 2, newid, cap)
    _pack_half(w, lo_deg, hi_deg, order[1::2], NT // 2, NT // 2, newid, cap)

    src = newid[src0]
    dst = newid[dst0]
    E = src.shape[0]

    tile_of_edge = dst >> 7
    side = (src >= NPH).astype(np.int64)
    key = tile_of_edge * 2 + side
    cnt = np.bincount(key, minlength=NT * 2)
    K2 = max(1, int(np.max(-(-cnt // P))))
    KT = 2 * K2
    order_e = np.argsort(key, kind="stable")
    bounds = np.concatenate([[0], np.cumsum(cnt)])
    within = np.arange(E) - bounds[key[order_e]]

    ks, ke = key[order_e], order_e
    # pad slots gather a zero row (a padding node's row) from each half
    padmask = np.ones(NP_, bool)
    padmask[newid[:N]] = False
    pad_rows = np.nonzero(padmask)[0]                   # new ids of pad nodes
    pad_lo = pad_rows[pad_rows < NPH][0]
    pad_hi = pad_rows[pad_rows >= NPH][0]
    IX = np.empty((NT, 2, K2 * P), np.int64)            # gather idx per side
    IX[:, 0, :] = _rho(pad_lo)
    IX[:, 1, :] = _rho(pad_hi) - NPH
    DSTLf = np.full((NT, KT * P), -1.0, np.float32)
    rho_src = _rho(src) - side * NPH
    IX[ks >> 1, ks & 1, within] = rho_src[ke]
    DSTLf[ks >> 1, (ks & 1) * K2 * P + within] = (dst[ke] & 127)
    assert IX.max() < 2 ** 15

    def per_core_wrap(a3):
        # [NT, K2*P] flat idx lists -> per-core [128, TPC*K2*8] int16 wrapped
        out = []
        for c in range(NCORES):
            a = a3[c * TPC:(c + 1) * TPC]               # [TPC, K2*P]
            wpd = np.transpose(a.reshape(TPC, K2 * 8, 16), (2, 0, 1))
            wpd = wpd.reshape(16, TPC * K2 * 8)
            out.append(np.tile(wpd, (8, 1)).astype(np.int16))
        return out

    IXL_cores = per_core_wrap(IX[:, 0, :])
    IXH_cores = per_core_wrap(IX[:, 1, :])

    def wrap_flat(flat):
        # flat idx list (len multiple of 16) -> [128, len/16] int16 wrapped
        a = flat.reshape(-1, 16).T                      # [16, len/16]
        return np.tile(a, (8, 1)).astype(np.int16)

    DSTL_cores = []
    OWNL_cores = []
    OWNH_cores = []
    for c in range(NCORES):
        a = DSTLf[c * TPC:(c + 1) * TPC].reshape(TPC * KT, P)
        DSTL_cores.append(np.ascontiguousarray(a.T).astype(bf))
        own = np.arange(TPC * P) + c * TPC * P          # own new ids, flat
        in_lo = own[0] < NPH                            # whole core same half
        ownl = _rho(own) if in_lo else np.full(TPC * P, _rho(pad_lo))
        ownh = (_rho(own) - NPH if not in_lo
                else np.full(TPC * P, _rho(pad_hi) - NPH))
        OWNL_cores.append(wrap_flat(ownl))
        OWNH_cores.append(wrap_flat(ownh))

    def wcat(W, att_s, att_d, ch):
        As = np.zeros((HEADS * ch, HEADS), np.float32)
        Ad = np.zeros((HEADS * ch, HEADS), np.float32)
        for h in range(HEADS):
            As[h * ch:(h + 1) * ch, h] = att_s[h]
            Ad[h * ch:(h + 1) * ch, h] = att_d[h]
        WT = W.T.astype(np.float32)
        cat = np.concatenate([WT, WT @ As, WT @ Ad], axis=1)
        perm = np.arange(HEADS * ch).reshape(HEADS, ch).T.ravel()
        cat[:, :HEADS * ch] = cat[:, perm]
        return cat

    assert not np.any(b1), "pad-row zero trick requires b1 == 0"
    W1cat = np.ascontiguousarray(wcat(W1, att_src1, att_dst1, HID)).astype(bf)
    W2cat = np.ascontiguousarray(wcat(W2, att_src2, att_dst2, OUT_F)).astype(bf)

    x_new = np.zeros((NP_, IN_F), np.float32)
    x_new[newid[:N]] = x
    xT_cores = [np.ascontiguousarray(
        x_new[c * TPC * P:(c + 1) * TPC * P].T).astype(bf)
        for c in range(NCORES)]

    IOTA = np.broadcast_to(np.arange(P, dtype=bf), (P, P)).copy()
    IDENT = np.eye(P, dtype=bf)
    B1bc = np.broadcast_to(b1.astype(np.float32), (P, HID)).copy()
    B2bc = np.broadcast_to(b2.astype(np.float32), (P, OUT_F)).copy()

    shapes = dict(N=N, IN_F=IN_F, HEADS=HEADS, HID=HID, OUT_F=OUT_F,
                  NP=NP_, NT=NT, TPC=TPC, K2=K2)
    shared = dict(W1cat=W1cat, W2cat=W2cat, IOTA=IOTA, IDENT=IDENT,
                  B1bc=B1bc, B2bc=B2bc, newid=newid)
    percore = [dict(xT=xT_cores[i], IXL=IXL_cores[i], IXH=IXH_cores[i],
                    DSTL=DSTL_cores[i], OWNL=OWNL_cores[i],
                    OWNH=OWNH_cores[i])
               for i in range(NCORES)]
    return shapes, shared, percore


def _build(s, qmap=None):
    import concourse.bass as bass
    import concourse.mybir as mybir
    import concourse.tile as tile
    from concourse import bacc, library_config

    f32 = mybir.dt.float32
    bf16 = mybir.dt.float16
    i32 = mybir.dt.int32
    i16 = mybir.dt.int16
    HEADS, HID, OUT_F, IN_F = s["HEADS"], s["HID"], s["OUT_F"], s["IN_F"]
    NP_, NT, TPC, K2 = s["NP"], s["NT"], s["TPC"], s["K2"]
    NPH = NP_ // 2
    KT = 2 * K2
    NH = HEADS * HID                    # 512
    GW = NH + 2 * HEADS                 # 520 (written)
    GW2 = 640                           # row stride (1280B, for dma_gather)
    NI = K2 * P                         # idxs per gather
    ICT = K2 * 8                        # idx cols per tile per side
    NCH = TPC * KT
    KC1 = IN_F // P
    AluOp = mybir.AluOpType
    Act = mybir.ActivationFunctionType

    nc = bacc.Bacc("TRN2", target_bir_lowering=False, debug=False,
                   num_devices=NCORES, num_swdge_queues=4)

    t_xT = nc.dram_tensor("xT", [IN_F, TPC * P], bf16, kind="ExternalInput")
    t_w1 = nc.dram_tensor("W1cat", [IN_F, GW], bf16, kind="ExternalInput")
    t_w2 = nc.dram_tensor("W2cat", [HID, GW], bf16, kind="ExternalInput")
    t_iota = nc.dram_tensor("IOTA", [P, P], bf16, kind="ExternalInput")
    t_ident = nc.dram_tensor("IDENT", [P, P], bf16, kind="ExternalInput")
    t_b1 = nc.dram_tensor("B1bc", [P, HID], f32, kind="ExternalInput")
    t_b2 = nc.dram_tensor("B2bc", [P, OUT_F], f32, kind="ExternalInput")
    t_ixl = nc.dram_tensor("IXL", [P, TPC * ICT], i16, kind="ExternalInput")
    t_ixh = nc.dram_tensor("IXH", [P, TPC * ICT], i16, kind="ExternalInput")
    t_dstl = nc.dram_tensor("DSTL", [P, NCH], bf16, kind="ExternalInput")
    t_ownl = nc.dram_tensor("OWNL", [P, TPC * 8], i16, kind="ExternalInput")
    t_ownh = nc.dram_tensor("OWNH", [P, TPC * 8], i16, kind="ExternalInput")
    t_out = nc.dram_tensor("out", [TPC * P, OUT_F], f32, kind="ExternalOutput")

    with tile.TileContext(nc) as tc:
        nc.gpsimd.load_library(library_config.mlp)
        with tc.tile_pool(name="const", bufs=1) as constp, \
             tc.tile_pool(name="dram", bufs=1, space="DRAM") as dramp, \
             tc.tile_pool(name="stage", bufs=3) as stagep, \
             tc.tile_pool(name="gat", bufs=3) as gatp, \
             tc.tile_pool(name="msk", bufs=2) as mskp, \
             tc.tile_pool(name="mskT", bufs=2) as mskTp, \
             tc.tile_pool(name="small", bufs=8) as smallp, \
             tc.tile_pool(name="accs", bufs=3) as accp:

            G1o = dramp.tile([TPC * P, GW2], bf16, name="G1o")
            G2o = dramp.tile([TPC * P, GW2], bf16, name="G2o")
            G1 = dramp.tile([NP_, GW2], bf16, name="G1",
                            addr_space="Shared")
            G2 = dramp.tile([NP_, GW2], bf16, name="G2",
                            addr_space="Shared")

            iota_sb = constp.tile([P, P], bf16, name="iota_sb")
            nc.sync.dma_start(out=iota_sb[:], in_=t_iota[:, :])
            ident_sb = constp.tile([P, P], bf16, name="ident_sb")
            nc.sync.dma_start(out=ident_sb[:], in_=t_ident[:, :])
            b1_sb = constp.tile([P, HID], f32, name="b1_sb")
            nc.sync.dma_start(out=b1_sb[:], in_=t_b1[:, :])
            b2_sb = constp.tile([P, OUT_F], f32, name="b2_sb")
            nc.sync.dma_start(out=b2_sb[:], in_=t_b2[:, :])
            w1_sb = []
            for k in range(KC1):
                w1k = constp.tile([P, GW], bf16, name=f"w1_sb{k}")
                nc.sync.dma_start(out=w1k[:], in_=t_w1[k * P:(k + 1) * P, :])
                w1_sb.append(w1k)
            w2_sb = constp.tile([P, GW], bf16, name="w2_sb")
            nc.sync.dma_start(out=w2_sb[:], in_=t_w2[:, :])
            ixl_sb = constp.tile([P, TPC * ICT], i16, name="ixl_sb")
            nc.sync.dma_start(out=ixl_sb[:], in_=t_ixl[:, :])
            ixh_sb = constp.tile([P, TPC * ICT], i16, name="ixh_sb")
            nc.sync.dma_start(out=ixh_sb[:], in_=t_ixh[:, :])
            dstl_sb = constp.tile([P, NCH], bf16, name="dstl_sb")
            nc.sync.dma_start(out=dstl_sb[:], in_=t_dstl[:, :])
            ownl_sb = constp.tile([P, TPC * 8], i16, name="ownl_sb")
            nc.sync.dma_start(out=ownl_sb[:], in_=t_ownl[:, :])
            ownh_sb = constp.tile([P, TPC * 8], i16, name="ownh_sb")
            nc.sync.dma_start(out=ownh_sb[:], in_=t_ownh[:, :])
            x_sb = []
            for k in range(KC1):
                xk = constp.tile([P, TPC * P], bf16, name=f"x_sb{k}")
                nc.sync.dma_start(out=xk[:],
                                  in_=t_xT[k * P:(k + 1) * P, :])
                x_sb.append(xk)
            hT_sb = constp.tile([P, TPC * P], bf16, name="hT_sb")

            def phase_a(Go, w_rhs, lhsT_tiles):
                # compute this core's own G rows (2-tile row interleave)
                kc = len(w_rhs)
                assert TPC % 2 == 0
                with tc.tile_pool(name="psA", bufs=3, space="PSUM") as psA:
                    for T0 in range(0, TPC, 2):
                        stg = stagep.tile([P, 2, GW], bf16, name="stg")
                        for q in range(2):
                            T = T0 + q
                            ps = psA.tile([P, GW], f32, name="aps")
                            for k in range(kc):
                                nc.tensor.matmul(
                                    ps[:, 0:NH],
                                    lhsT=lhsT_tiles[k][:, T * P:(T + 1) * P],
                                    rhs=w_rhs[k][:, 0:NH],
                                    start=(k == 0), stop=(k == kc - 1))
                                nc.tensor.matmul(
                                    ps[:, NH:GW],
                                    lhsT=lhsT_tiles[k][:, T * P:(T + 1) * P],
                                    rhs=w_rhs[k][:, NH:GW],
                                    start=(k == 0), stop=(k == kc - 1))
                            nc.scalar.activation(out=stg[:, q, :],
                                                 in_=ps[:, :], func=Act.Copy)
                        dst = Go[T0 * P:(T0 + 2) * P, 0:GW].rearrange(
                            "(p q) w -> p q w", q=2)
                        nc.sync.dma_start(out=dst, in_=stg[:])

            qc = [0]
            gcalls = nc._gat_insts = []

            def nextq():
                q = qmap[qc[0]] if qmap is not None else 0
                qc[0] += 1
                return q

            def phase_b(G, Go, bbc_sb, writer):
              with tc.tile_pool(name="psB", bufs=2, space="PSUM") as psB, \
                   tc.tile_pool(name="psC", bufs=1, space="PSUM") as psC, \
                   tc.tile_pool(name="psT", bufs=1, space="PSUM") as psTp, \
                   tc.tile_pool(name="adp", bufs=1) as adpool:
                # own-node a_d for all tiles: gather 128-col blocks at
                # G[row, 512:640] from both halves (wrong half reads a pad
                # node's zero row), then add.  10 tiles per gather.
                TG = next(d for d in range(min(10, TPC), 0, -1)
                          if TPC % d == 0)
                adL = adpool.tile([P, TPC, P], bf16, name="adL")
                adH = adpool.tile([P, TPC, P], bf16, name="adH")
                for i in range(TPC // TG):
                    for buf, ix in ((adL, ownl_sb), (adH, ownh_sb)):
                        gcalls.append(nc.gpsimd.dma_gather(
                            out_ap=buf[:, i * TG:(i + 1) * TG, :],
                            in_ap=G[0:NPH, 512:GW2] if buf is adL
                            else G[NPH:NP_, 512:GW2],
                            idxs_ap=ix[:, i * TG * 8:(i + 1) * TG * 8],
                            num_idxs=TG * P, num_idxs_reg=TG * P,
                            elem_size=P, elem_step=GW2,
                            single_packet=False, queue_num=nextq()))
                adS = adpool.tile([P, TPC, HEADS], bf16, name="adS")
                nc.vector.tensor_tensor(out=adS[:], in0=adL[:, :, 4:8],
                                        in1=adH[:, :, 4:8], op=AluOp.add)
                Gov = Go[:, :].rearrange("(b p q) w -> q b p w", q=2, p=P)
                for t in range(TPC):
                    # gather all KT*128 source rows of this tile
                    g = gatp.tile([P, KT + 1, GW2], bf16, name="g")
                    nc.sync.dma_start(out=g[:, KT, 0:GW],
                                      in_=Gov[t & 1, t >> 1, :, 0:GW])
                    gcalls.append(nc.gpsimd.dma_gather(
                        out_ap=g[:, 0:K2, :], in_ap=G[0:NPH, :],
                        idxs_ap=ixl_sb[:, t * ICT:(t + 1) * ICT],
                        num_idxs=NI, num_idxs_reg=NI, elem_size=GW2,
                        single_packet=False, queue_num=nextq()))
                    gcalls.append(nc.gpsimd.dma_gather(
                        out_ap=g[:, K2:KT, :], in_ap=G[NPH:NP_, :],
                        idxs_ap=ixh_sb[:, t * ICT:(t + 1) * ICT],
                        num_idxs=NI, num_idxs_reg=NI, elem_size=GW2,
                        single_packet=False, queue_num=nextq()))
                    # all KT edge->dst one-hot masks in one DVE op
                    mask0 = mskp.tile([P, KT, P], bf16, name="mask0")
                    nc.vector.tensor_tensor(
                        out=mask0[:],
                        in0=iota_sb[:, None, :].broadcast_to([P, KT, P]),
                        in1=dstl_sb[:, t * KT:(t + 1) * KT, None]
                            .broadcast_to([P, KT, P]),
                        op=AluOp.is_equal)
                    # transpose masks (batches of 8 per PSUM bank)
                    maskT = mskTp.tile([P, KT, P], bf16, name="maskT")
                    for b0 in range(0, KT, 8):
                        nb = min(8, KT - b0)
                        psT = psTp.tile([P, 8, P], bf16, name="psTt")
                        for j in range(nb):
                            nc.tensor.transpose(psT[:, j, :],
                                                mask0[:, b0 + j, :],
                                                ident_sb[:])
                        nc.scalar.activation(out=maskT[:, b0:b0 + nb, :],
                                             in_=psT[:, 0:nb, :],
                                             func=Act.Copy)
                    # per-edge a_d via KT tiny matmuls
                    adpe = psC.tile([P, KT * HEADS], f32, name="adpe")
                    for k in range(KT):
                        nc.tensor.matmul(adpe[:, k * HEADS:(k + 1) * HEADS],
                                         lhsT=maskT[:, k, :],
                                         rhs=adS[:, t, :],
                                         start=True, stop=True)
                    # alpha = a_s + a_d ; leaky relu ; exp
                    alpha = smallp.tile([P, KT + 1, HEADS], f32, name="alpha")
                    nc.vector.tensor_tensor(
                        out=alpha[:, 0:KT, :], in0=g[:, 0:KT, NH:NH + HEADS],
                        in1=adpe[:].rearrange("p (k h) -> p k h", h=HEADS),
                        op=AluOp.add)
                    nc.vector.tensor_tensor(
                        out=alpha[:, KT, :], in0=g[:, KT, NH:NH + HEADS],
                        in1=adS[:, t, :], op=AluOp.add)
                    t2 = smallp.tile([P, KT + 1, HEADS], f32, name="t2")
                    nc.vector.tensor_scalar_mul(t2[:], alpha[:], 0.2)
                    nc.vector.tensor_tensor(out=alpha[:], in0=alpha[:],
                                            in1=t2[:], op=AluOp.max)
                    ex = smallp.tile([P, KT + 1, HEADS], bf16, name="ex")
                    nc.scalar.activation(out=ex[:], in_=alpha[:],
                                         func=Act.Exp)
                    # scale all messages in place: g[:, :, h*HID:...] *= ex_h
                    gv = g[:, :, 0:NH].rearrange("p k (c h) -> p k c h",
                                                 h=HEADS)
                    nc.vector.tensor_tensor(
                        out=gv, in0=gv,
                        in1=ex[:, :, None, :].broadcast_to(
                            [P, KT + 1, HID, HEADS]),
                        op=AluOp.mult)
                    # scatter: out += mask.T @ msg ; den += mask.T @ ex
                    out_ps = psB.tile([P, NH], f32, name="outps")
                    den_ps = psB.tile([P, HEADS], f32, name="denps")
                    for k in range(KT + 1):
                        lhs = ident_sb[:] if k == KT else mask0[:, k, :]
                        first, last = (k == 0), (k == KT)
                        nc.tensor.matmul(out_ps[:, :], lhsT=lhs,
                                         rhs=g[:, k, 0:NH],
                                         start=first, stop=last)
                        nc.tensor.matmul(den_ps[:, :], lhsT=lhs,
                                         rhs=ex[:, k, :],
                                         start=first, stop=last)
                    # h = sum_h out_h / (4*den_h) + b
                    den4 = smallp.tile([P, HEADS], f32, name="den4")
                    nc.vector.tensor_scalar(
                        out=den4[:], in0=den_ps[:], scalar1=float(HEADS),
                        scalar2=float(HEADS) * 1e-16, op0=AluOp.mult,
                        op1=AluOp.add)
                    rec = smallp.tile([P, HEADS], f32, name="rec")
                    nc.vector.reciprocal(rec[:], den4[:])
                    tmp = accp.tile([P, HID, HEADS], f32, name="tmpacc")
                    opv = out_ps[:].rearrange("p (c h) -> p c h", h=HEADS)
                    for h in range(HEADS):
                        nc.scalar.activation(out=tmp[:, :, h],
                                             in_=opv[:, :, h], func=Act.Copy,
                                             scale=rec[:, h:h + 1])
                    acc = accp.tile([P, HID], f32, name="acc")
                    nc.vector.tensor_tensor(out=tmp[:, :, 0:2],
                                            in0=tmp[:, :, 0:2],
                                            in1=tmp[:, :, 2:4], op=AluOp.add)
                    nc.vector.tensor_tensor(out=tmp[:, :, 0],
                                            in0=tmp[:, :, 0],
                                            in1=tmp[:, :, 1], op=AluOp.add)
                    nc.vector.tensor_tensor(out=acc[:], in0=tmp[:, :, 0],
                                            in1=bbc_sb[:], op=AluOp.add)
                    writer(t, acc, psTp)

            def write_h(t, acc, psTp):
                accb = accp.tile([P, HID], bf16, name="accb")
                nc.scalar.activation(out=accb[:], in_=acc[:], func=Act.Copy)
                tp = psTp.tile([P, 8, P], bf16, name="psTt")
                nc.tensor.transpose(tp[:, 0, :], accb[:], ident_sb[:])
                nc.scalar.activation(out=hT_sb[:, t * P:(t + 1) * P],
                                     in_=tp[:, 0, :], func=Act.Copy)

            def write_out(t, acc, psTp):
                nc.sync.dma_start(out=t_out[t * P:(t + 1) * P, :], in_=acc[:])

            def ag(Go, Gf):
                nc.gpsimd.collective_compute(
                    "AllGather", AluOp.bypass,
                    replica_groups=[list(range(NCORES))],
                    ins=[Go[:, :].rearrange("r w -> (r w)").unsqueeze(0)
                         .opt()],
                    outs=[Gf[:, :].rearrange("(o r) w -> o (r w)",
                                             o=NCORES).opt()])

            def ag_chunk(Go_, Gf, row0, row1):
                nc.gpsimd.collective_compute(
                    "AllGather", AluOp.bypass,
                    replica_groups=[list(range(NCORES))],
                    ins=[Go_[row0:row1, :]
                         .rearrange("r w -> (r w)").unsqueeze(0).opt()],
                    outs=[Gf[:, :].rearrange("(o r) w -> o (r w)", o=NCORES)
                          [:, row0 * GW2:row1 * GW2].opt()])

            G2ov = G2o[:, :].rearrange("(b p q) w -> q b p w", q=2, p=P)
            psA2holder = []

            # layer-2 phase A for one tile (h tile just written to hT_sb)
            def phA2_tile(t):
                psA2 = psA2holder[0]
                ps = psA2.tile([P, GW], f32, name="a2ps")
                nc.tensor.matmul(
                    ps[:, 0:NH], lhsT=hT_sb[:, t * P:(t + 1) * P],
                    rhs=w2_sb[:, 0:NH], start=True, stop=True)
                nc.tensor.matmul(
                    ps[:, NH:GW], lhsT=hT_sb[:, t * P:(t + 1) * P],
                    rhs=w2_sb[:, NH:GW], start=True, stop=True)
                stg = stagep.tile([P, GW], bf16, name="stg2")
                nc.scalar.activation(out=stg[:], in_=ps[:, :], func=Act.Copy)
                nc.sync.dma_start(out=G2ov[t & 1, t >> 1, :, 0:GW],
                                  in_=stg[:])

            AGCH = TPC          # single AG2 (Shared DRAM: one writer)

            def write_h2(t, acc, psTp):
                write_h(t, acc, psTp)
                phA2_tile(t)
                if (t + 1) % AGCH == 0 and (t & 1) == 1:
                    ag_chunk(G2o, G2, (t + 1 - AGCH) * P, (t + 1) * P)

            import os
            PH = int(os.environ.get("KPH", "5"))
            with nc.named_scope("phA1"):
                phase_a(G1o, w1_sb, x_sb)
            with nc.named_scope("phAG1"):
                ag(G1o, G1)
            if PH >= 2:
                with nc.named_scope("phB1"), \
                     tc.tile_pool(name="psA2", bufs=1, space="PSUM") as psA2p:
                    psA2holder.append(psA2p)
                    phase_b(G1, G1o, b1_sb,
                            write_h2 if PH >= 4 else write_h)
            if PH >= 5:
                with nc.named_scope("phB2"):
                    phase_b(G2, G2o, b2_sb, write_out)

    nc.compile()
    return nc


def _sched_gather_info(nc):
    """Final-order scheduled positions of InstDMAGatherAnt, name -> pos."""
    pos = {}
    i = 0
    for blk in nc.m.functions[0].blocks:
        for inst in blk.instructions:
            if type(inst).__name__ == "InstDMAGatherAnt":
                pos[inst.name] = i
                i += 1
    return pos


def _queue_consistent(nc):
    """Each DMA semaphore must be incremented by exactly one SWDGE queue."""
    sem_q = {}
    for blk in nc.m.functions[0].blocks:
        for inst in blk.instructions:
            if type(inst).__name__ != "InstDMAGatherAnt":
                continue
            si = inst.sync_info
            if not si or not si.on_update:
                continue
            sem = si.on_update[0].id
            q = inst.queue_num
            if sem_q.setdefault(sem, q) != q:
                return False
    return True


def _build_multiq(s):
    nc = _build(s, None)
    names = [g.ins.name for g in nc._gat_insts]
    for _ in range(3):
        pos = _sched_gather_info(nc)
        qmap = [pos[n] % 4 for n in names]
        nc2 = _build(s, qmap)
        if _queue_consistent(nc2):
            return nc2
        nc = nc2
        names = [g.ins.name for g in nc._gat_insts]
    return _build(s, None)                               # safe fallback


def _get_nc(s):
    key = tuple(sorted(s.items()))
    if key not in _CACHE:
        _CACHE[key] = _build_multiq(s)
    return _CACHE[key]


def _in_maps(shared, percore):
    maps = []
    for i in range(NCORES):
        maps.append({"xT": percore[i]["xT"], "W1cat": shared["W1cat"],
                     "W2cat": shared["W2cat"], "IOTA": shared["IOTA"],
                     "IDENT": shared["IDENT"], "B1bc": shared["B1bc"],
                     "B2bc": shared["B2bc"], "IXL": percore[i]["IXL"],
                     "IXH": percore[i]["IXH"], "DSTL": percore[i]["DSTL"],
                     "OWNL": percore[i]["OWNL"],
                     "OWNH": percore[i]["OWNH"]})
    return maps


def kernel(**inputs):
    from concourse import bass_utils

    x = np.asarray(inputs["x"], dtype=np.float32)
    edge_index = np.asarray(inputs["edge_index"])
    args = (x, edge_index,
            np.asarray(inputs["W1"], np.float32),
            np.asarray(inputs["att_src1"], np.float32),
            np.asarray(inputs["att_dst1"], np.float32),
            np.asarray(inputs["b1"], np.float32),
            np.asarray(inputs["W2"], np.float32),
            np.asarray(inputs["att_src2"], np.float32),
            np.asarray(inputs["att_dst2"], np.float32),
            np.asarray(inputs["b2"], np.float32))
    shapes, shared, percore = _host_prep(*args)
    nc = _get_nc(shapes)
    res = bass_utils.run_bass_kernel_spmd(nc, _in_maps(shared, percore),
                                          core_ids=list(range(NCORES)))
    out_cat = np.concatenate(
        [res.results[i]["out"] for i in range(NCORES)], axis=0)
    out = out_cat[shared["newid"][:shapes["N"]]]
    return np.ascontiguousarray(out, dtype=np.float32)


# revision 24
# speedup vs baseline: 1.0240x; 1.0240x over previous
"""2-layer GAT (GATConv x2, mean over 4 heads) on 8 Trainium2 NeuronCores.

Strategy (dst-segment parallel, per-tile batched dma_gather):
  - Host: self-loops are appended as ordinary edges.  Nodes are split into
    lo/hi halves of 25600 (so gather indices fit int16), each half packed
    by greedy (in-degree+1) bin-packing into 200 tiles of 128 (50 tiles
    per core total).  Each core owns all edges of its 50 tiles, so the
    segment softmax never crosses cores.
  - G rows (fp16, 1280B stride for dma_gather): [xh(512)|a_s(4)|a_d(4)|pad],
    4-way interleaved within 4-tile groups so phase-A stores move 4 rows
    per DMA descriptor.
  - Phase B per tile (KT = 2*K2 chunks = KT*128 edge slots):
      * TWO dma_gathers (InstDMAGatherAnt, single_packet=False) fetch all
        source rows: tile edges are grouped by src half, lo edges in
        chunks [0,K2), hi in [K2,KT); pad slots gather row 0 (masked out
        later via dstl=-1),
      * ONE [128,1]-offset indirect DMA fetches the tile's own a_d vector
        straight out of G via element_offset=516,
      * ONE DVE op builds all KT edge->dst one-hot masks (broadcast APs),
      * per-edge a_d = maskT.T @ ad_tile: masks are PE-transposed in
        batches of 8 per PSUM bank, then KT 4-col matmuls,
      * alpha = a_s + a_d, leaky-relu on DVE, exp on the Scalar engine
        (softmax max-shift skipped: alpha is O(1), softmax shift-invariant),
      * ONE in-place DVE op scales all messages by ex,
      * per chunk 2 PE matmuls: out += mask.T @ msg (512 col) and
        den += mask.T @ ex (4 col), accumulating in separate PSUM banks.
    Head-mean + 1/den normalization is a handful of batched DVE ops.
  - h tiles are PE-transposed, AllGather'd in fp16, and consumed as
    ready-made lhsT slabs by layer-2's phase A.
"""

import numpy as np

P = 128
NCORES = 8

_CACHE = {}


def _pack_half(weight, lo_deg, hi_deg, nodes, tile0, NTH, newid, cap):
    """Greedy largest-first packing of `nodes` into NTH tiles of 128,
    keeping each tile's lo-source and hi-source edge counts under cap."""
    import heapq
    heap = [(0, tile0 + t) for t in range(NTH)]
    heapq.heapify(heap)
    fill = {tile0 + t: 0 for t in range(NTH)}
    lo = {tile0 + t: 0 for t in range(NTH)}
    hi = {tile0 + t: 0 for t in range(NTH)}
    for node in nodes:                       # pre-sorted by weight desc
        popped = []
        placed = None
        while heap:
            cnt, t = heapq.heappop(heap)
            if (lo[t] + lo_deg[node] <= cap
                    and hi[t] + hi_deg[node] <= cap):
                placed = (cnt, t)
                break
            popped.append((cnt, t))
        if placed is None:                   # overflow: least-loaded tile
            placed = popped.pop(0)
        for item in popped:
            heapq.heappush(heap, item)
        cnt, t = placed
        newid[node] = t * P + fill[t]
        fill[t] += 1
        lo[t] += lo_deg[node]
        hi[t] += hi_deg[node]
        if fill[t] < P:
            heapq.heappush(heap, (int(cnt + weight[node]), t))


def _rho(n):
    """node-id -> G-row-id: 2-way interleave within groups of 2 tiles
    (so phase-A stores move 2 rows per descriptor).  Preserves 256-blocks,
    so lo/hi halves and per-core blocks map to themselves."""
    return ((n >> 8) << 8) | ((n & 127) << 1) | ((n >> 7) & 1)


def _host_prep(x, edge_index, W1, att_src1, att_dst1, b1, W2, att_src2,
               att_dst2, b2):
    bf = np.float16

    N, IN_F = x.shape
    HEADS, HID = att_src1.shape
    OUT_F = att_src2.shape[1]
    TPC = -(-int(N * 1.02) // (NCORES * P))   # ~2% slack for packing
    NT = NCORES * TPC
    NP_ = NT * P
    NPH = NP_ // 2
    assert NP_ >= N and NT % 2 == 0

    src0 = np.asarray(edge_index[0], np.int64)
    dst0 = np.asarray(edge_index[1], np.int64)
    w = np.zeros(NP_, np.int64)
    w[:N] = np.bincount(dst0, minlength=N)              # self loop separate

    # split nodes into lo/hi halves (snake over weight-desc order), pack each
    order = np.argsort(-w, kind="stable")
    half = np.empty(NP_, np.int8)
    half[order[0::2]] = 0
    half[order[1::2]] = 1
    lo_deg = np.zeros(NP_, np.int64)
    np.add.at(lo_deg, dst0, (half[src0] == 0))
    hi_deg = w - lo_deg
    ssum = max(lo_deg[half == 0].sum(), hi_deg[half == 0].sum(),
               lo_deg[half == 1].sum(), hi_deg[half == 1].sum())
    cap = int(np.ceil(ssum / (NT // 2) / P)) * P
    newid = np.empty(NP_, np.int64)
    _pack_half(w, lo_deg, hi_deg, order[0::2], 0, NT // 2, newid, cap)
    _pack_half(w, lo_deg, hi_deg, order[1::2], NT // 2, NT // 2, newid, cap)

    src = newid[src0]
    dst = newid[dst0]
    E = src.shape[0]

    tile_of_edge = dst >> 7
    side = (src >= NPH).astype(np.int64)
    key = tile_of_edge * 2 + side
    cnt = np.bincount(key, minlength=NT * 2)
    K2 = max(1, int(np.max(-(-cnt // P))))
    KT = 2 * K2
    order_e = np.argsort(key, kind="stable")
    bounds = np.concatenate([[0], np.cumsum(cnt)])
    within = np.arange(E) - bounds[key[order_e]]

    ks, ke = key[order_e], order_e
    # pad slots gather a zero row (a padding node's row) from each half
    padmask = np.ones(NP_, bool)
    padmask[newid[:N]] = False
    pad_rows = np.nonzero(padmask)[0]                   # new ids of pad nodes
    pad_lo = pad_rows[pad_rows < NPH][0]
    pad_hi = pad_rows[pad_rows >= NPH][0]
    IX = np.empty((NT, 2, K2 * P), np.int64)            # gather idx per side
    IX[:, 0, :] = _rho(pad_lo)
    IX[:, 1, :] = _rho(pad_hi) - NPH
    DSTLf = np.full((NT, KT * P), -1.0, np.float32)
    rho_src = _rho(src) - side * NPH
    IX[ks >> 1, ks & 1, within] = rho_src[ke]
    DSTLf[ks >> 1, (ks & 1) * K2 * P + within] = (dst[ke] & 127)
    assert IX.max() < 2 ** 15

    def per_core_wrap(a3):
        # [NT, K2*P] flat idx lists -> per-core [128, TPC*K2*8] int16 wrapped
        out = []
        for c in range(NCORES):
            a = a3[c * TPC:(c + 1) * TPC]               # [TPC, K2*P]
            wpd = np.transpose(a.reshape(TPC, K2 * 8, 16), (2, 0, 1))
            wpd = wpd.reshape(16, TPC * K2 * 8)
            out.append(np.tile(wpd, (8, 1)).astype(np.int16))
        return out

    IXL_cores = per_core_wrap(IX[:, 0, :])
    IXH_cores = per_core_wrap(IX[:, 1, :])

    def wrap_flat(flat):
        # flat idx list (len multiple of 16) -> [128, len/16] int16 wrapped
        a = flat.reshape(-1, 16).T                      # [16, len/16]
        return np.tile(a, (8, 1)).astype(np.int16)

    DSTL_cores = []
    OWNL_cores = []
    OWNH_cores = []
    for c in range(NCORES):
        a = DSTLf[c * TPC:(c + 1) * TPC].reshape(TPC * KT, P)
        DSTL_cores.append(np.ascontiguousarray(a.T).astype(bf))
        own = np.arange(TPC * P) + c * TPC * P          # own new ids, flat
        in_lo = own[0] < NPH                            # whole core same half
        ownl = _rho(own) if in_lo else np.full(TPC * P, _rho(pad_lo))
        ownh = (_rho(own) - NPH if not in_lo
                else np.full(TPC * P, _rho(pad_hi) - NPH))
        OWNL_cores.append(wrap_flat(ownl))
        OWNH_cores.append(wrap_flat(ownh))

    def wcat(W, att_s, att_d, ch):
        As = np.zeros((HEADS * ch, HEADS), np.float32)
        Ad = np.zeros((HEADS * ch, HEADS), np.float32)
        for h in range(HEADS):
            As[h * ch:(h + 1) * ch, h] = att_s[h]
            Ad[h * ch:(h + 1) * ch, h] = att_d[h]
        WT = W.T.astype(np.float32)
        cat = np.concatenate([WT, WT @ As, WT @ Ad], axis=1)
        perm = np.arange(HEADS * ch).reshape(HEADS, ch).T.ravel()
        cat[:, :HEADS * ch] = cat[:, perm]
        return cat

    assert not np.any(b1), "pad-row zero trick requires b1 == 0"
    W1cat = np.ascontiguousarray(wcat(W1, att_src1, att_dst1, HID)).astype(bf)
    W2cat = np.ascontiguousarray(wcat(W2, att_src2, att_dst2, OUT_F)).astype(bf)

    x_new = np.zeros((NP_, IN_F), np.float32)
    x_new[newid[:N]] = x
    xT_cores = [np.ascontiguousarray(
        x_new[c * TPC * P:(c + 1) * TPC * P].T).astype(bf)
        for c in range(NCORES)]

    IOTA = np.broadcast_to(np.arange(P, dtype=bf), (P, P)).copy()
    IDENT = np.eye(P, dtype=bf)
    B1bc = np.broadcast_to(b1.astype(np.float32), (P, HID)).copy()
    B2bc = np.broadcast_to(b2.astype(np.float32), (P, OUT_F)).copy()

    shapes = dict(N=N, IN_F=IN_F, HEADS=HEADS, HID=HID, OUT_F=OUT_F,
                  NP=NP_, NT=NT, TPC=TPC, K2=K2)
    shared = dict(W1cat=W1cat, W2cat=W2cat, IOTA=IOTA, IDENT=IDENT,
                  B1bc=B1bc, B2bc=B2bc, newid=newid)
    percore = [dict(xT=xT_cores[i], IXL=IXL_cores[i], IXH=IXH_cores[i],
                    DSTL=DSTL_cores[i], OWNL=OWNL_cores[i],
                    OWNH=OWNH_cores[i])
               for i in range(NCORES)]
    return shapes, shared, percore


def _build(s, qmap=None):
    import concourse.bass as bass
    import concourse.mybir as mybir
    import concourse.tile as tile
    from concourse import bacc, library_config

    f32 = mybir.dt.float32
    bf16 = mybir.dt.float16
    i32 = mybir.dt.int32
    i16 = mybir.dt.int16
    HEADS, HID, OUT_F, IN_F = s["HEADS"], s["HID"], s["OUT_F"], s["IN_F"]
    NP_, NT, TPC, K2 = s["NP"], s["NT"], s["TPC"], s["K2"]
    NPH = NP_ // 2
    KT = 2 * K2
    NH = HEADS * HID                    # 512
    GW = NH + 2 * HEADS                 # 520 (written)
    GW2 = 640                           # row stride (1280B, for dma_gather)
    NI = K2 * P                         # idxs per gather
    ICT = K2 * 8                        # idx cols per tile per side
    NCH = TPC * KT
    KC1 = IN_F // P
    AluOp = mybir.AluOpType
    Act = mybir.ActivationFunctionType

    nc = bacc.Bacc("TRN2", target_bir_lowering=False, debug=False,
                   num_devices=NCORES, num_swdge_queues=4)

    t_xT = nc.dram_tensor("xT", [IN_F, TPC * P], bf16, kind="ExternalInput")
    t_w1 = nc.dram_tensor("W1cat", [IN_F, GW], bf16, kind="ExternalInput")
    t_w2 = nc.dram_tensor("W2cat", [HID, GW], bf16, kind="ExternalInput")
    t_iota = nc.dram_tensor("IOTA", [P, P], bf16, kind="ExternalInput")
    t_ident = nc.dram_tensor("IDENT", [P, P], bf16, kind="ExternalInput")
    t_b1 = nc.dram_tensor("B1bc", [P, HID], f32, kind="ExternalInput")
    t_b2 = nc.dram_tensor("B2bc", [P, OUT_F], f32, kind="ExternalInput")
    t_ixl = nc.dram_tensor("IXL", [P, TPC * ICT], i16, kind="ExternalInput")
    t_ixh = nc.dram_tensor("IXH", [P, TPC * ICT], i16, kind="ExternalInput")
    t_dstl = nc.dram_tensor("DSTL", [P, NCH], bf16, kind="ExternalInput")
    t_ownl = nc.dram_tensor("OWNL", [P, TPC * 8], i16, kind="ExternalInput")
    t_ownh = nc.dram_tensor("OWNH", [P, TPC * 8], i16, kind="ExternalInput")
    t_out = nc.dram_tensor("out", [TPC * P, OUT_F], f32, kind="ExternalOutput")

    with tile.TileContext(nc) as tc:
        nc.gpsimd.load_library(library_config.mlp)
        with tc.tile_pool(name="const", bufs=1) as constp, \
             tc.tile_pool(name="dram", bufs=1, space="DRAM") as dramp, \
             tc.tile_pool(name="stage", bufs=3) as stagep, \
             tc.tile_pool(name="gat", bufs=3) as gatp, \
             tc.tile_pool(name="msk", bufs=2) as mskp, \
             tc.tile_pool(name="mskT", bufs=2) as mskTp, \
             tc.tile_pool(name="small", bufs=8) as smallp, \
             tc.tile_pool(name="accs", bufs=3) as accp:

            G1o = dramp.tile([TPC * P, GW2], bf16, name="G1o")
            G2o = dramp.tile([TPC * P, GW2], bf16, name="G2o")
            G1 = dramp.tile([NP_, GW2], bf16, name="G1",
                            addr_space="Shared")
            G2 = dramp.tile([NP_, GW2], bf16, name="G2",
                            addr_space="Shared")

            iota_sb = constp.tile([P, P], bf16, name="iota_sb")
            nc.sync.dma_start(out=iota_sb[:], in_=t_iota[:, :])
            ident_sb = constp.tile([P, P], bf16, name="ident_sb")
            nc.sync.dma_start(out=ident_sb[:], in_=t_ident[:, :])
            b1_sb = constp.tile([P, HID], f32, name="b1_sb")
            nc.sync.dma_start(out=b1_sb[:], in_=t_b1[:, :])
            b2_sb = constp.tile([P, OUT_F], f32, name="b2_sb")
            nc.sync.dma_start(out=b2_sb[:], in_=t_b2[:, :])
            w1_sb = []
            for k in range(KC1):
                w1k = constp.tile([P, GW], bf16, name=f"w1_sb{k}")
                nc.sync.dma_start(out=w1k[:], in_=t_w1[k * P:(k + 1) * P, :])
                w1_sb.append(w1k)
            w2_sb = constp.tile([P, GW], bf16, name="w2_sb")
            nc.sync.dma_start(out=w2_sb[:], in_=t_w2[:, :])
            ixl_sb = constp.tile([P, TPC * ICT], i16, name="ixl_sb")
            nc.sync.dma_start(out=ixl_sb[:], in_=t_ixl[:, :])
            ixh_sb = constp.tile([P, TPC * ICT], i16, name="ixh_sb")
            nc.sync.dma_start(out=ixh_sb[:], in_=t_ixh[:, :])
            dstl_sb = constp.tile([P, NCH], bf16, name="dstl_sb")
            nc.sync.dma_start(out=dstl_sb[:], in_=t_dstl[:, :])
            ownl_sb = constp.tile([P, TPC * 8], i16, name="ownl_sb")
            nc.sync.dma_start(out=ownl_sb[:], in_=t_ownl[:, :])
            ownh_sb = constp.tile([P, TPC * 8], i16, name="ownh_sb")
            nc.sync.dma_start(out=ownh_sb[:], in_=t_ownh[:, :])
            x_sb = []
            for k in range(KC1):
                xk = constp.tile([P, TPC * P], bf16, name=f"x_sb{k}")
                nc.sync.dma_start(out=xk[:],
                                  in_=t_xT[k * P:(k + 1) * P, :])
                x_sb.append(xk)
            hT_sb = constp.tile([P, TPC * P], bf16, name="hT_sb")

            def phase_a(Go, w_rhs, lhsT_tiles):
                # compute this core's own G rows (2-tile row interleave)
                kc = len(w_rhs)
                assert TPC % 2 == 0
                with tc.tile_pool(name="psA", bufs=3, space="PSUM") as psA:
                    for T0 in range(0, TPC, 2):
                        stg = stagep.tile([P, 2, GW], bf16, name="stg")
                        for q in range(2):
                            T = T0 + q
                            ps = psA.tile([P, GW], f32, name="aps")
                            for k in range(kc):
                                nc.tensor.matmul(
                                    ps[:, 0:NH],
                                    lhsT=lhsT_tiles[k][:, T * P:(T + 1) * P],
                                    rhs=w_rhs[k][:, 0:NH],
                                    start=(k == 0), stop=(k == kc - 1))
                                nc.tensor.matmul(
                                    ps[:, NH:GW],
                                    lhsT=lhsT_tiles[k][:, T * P:(T + 1) * P],
                                    rhs=w_rhs[k][:, NH:GW],
                                    start=(k == 0), stop=(k == kc - 1))
                            nc.scalar.activation(out=stg[:, q, :],
                                                 in_=ps[:, :], func=Act.Copy)
                        dst = Go[T0 * P:(T0 + 2) * P, 0:GW].rearrange(
                            "(p q) w -> p q w", q=2)
                        nc.sync.dma_start(out=dst, in_=stg[:])

            qc = [0]
            gcalls = nc._gat_insts = []

            def nextq():
                q = qmap[qc[0]] if qmap is not None else 0
                qc[0] += 1
                return q

            def phase_b(G, Go, bbc_sb, writer):
              with tc.tile_pool(name="psB", bufs=2, space="PSUM") as psB, \
                   tc.tile_pool(name="psC", bufs=2, space="PSUM") as psC, \
                   tc.tile_pool(name="psT", bufs=1, space="PSUM") as psTp, \
                   tc.tile_pool(name="adp", bufs=1) as adpool:
                # own-node a_d for all tiles: gather 128-col blocks at
                # G[row, 512:640] from both halves (wrong half reads a pad
                # node's zero row), then add.  10 tiles per gather.
                TG = next(d for d in range(min(10, TPC), 0, -1)
                          if TPC % d == 0)
                adL = adpool.tile([P, TPC, P], bf16, name="adL")
                adH = adpool.tile([P, TPC, P], bf16, name="adH")
                for i in range(TPC // TG):
                    for buf, ix in ((adL, ownl_sb), (adH, ownh_sb)):
                        gcalls.append(nc.gpsimd.dma_gather(
                            out_ap=buf[:, i * TG:(i + 1) * TG, :],
                            in_ap=G[0:NPH, 512:GW2] if buf is adL
                            else G[NPH:NP_, 512:GW2],
                            idxs_ap=ix[:, i * TG * 8:(i + 1) * TG * 8],
                            num_idxs=TG * P, num_idxs_reg=TG * P,
                            elem_size=P, elem_step=GW2,
                            single_packet=False, queue_num=nextq()))
                adS = adpool.tile([P, TPC, HEADS], bf16, name="adS")
                nc.vector.tensor_tensor(out=adS[:], in0=adL[:, :, 4:8],
                                        in1=adH[:, :, 4:8], op=AluOp.add)
                Gov = Go[:, :].rearrange("(b p q) w -> q b p w", q=2, p=P)
                for t in range(TPC):
                    # gather all KT*128 source rows of this tile
                    g = gatp.tile([P, KT + 1, GW2], bf16, name="g")
                    nc.sync.dma_start(out=g[:, KT, 0:GW],
                                      in_=Gov[t & 1, t >> 1, :, 0:GW])
                    gcalls.append(nc.gpsimd.dma_gather(
                        out_ap=g[:, 0:K2, :], in_ap=G[0:NPH, :],
                        idxs_ap=ixl_sb[:, t * ICT:(t + 1) * ICT],
                        num_idxs=NI, num_idxs_reg=NI, elem_size=GW2,
                        single_packet=False, queue_num=nextq()))
                    gcalls.append(nc.gpsimd.dma_gather(
                        out_ap=g[:, K2:KT, :], in_ap=G[NPH:NP_, :],
                        idxs_ap=ixh_sb[:, t * ICT:(t + 1) * ICT],
                        num_idxs=NI, num_idxs_reg=NI, elem_size=GW2,
                        single_packet=False, queue_num=nextq()))
                    # all KT edge->dst one-hot masks in one DVE op
                    mask0 = mskp.tile([P, KT, P], bf16, name="mask0")
                    nc.vector.tensor_tensor(
                        out=mask0[:],
                        in0=iota_sb[:, None, :].broadcast_to([P, KT, P]),
                        in1=dstl_sb[:, t * KT:(t + 1) * KT, None]
                            .broadcast_to([P, KT, P]),
                        op=AluOp.is_equal)
                    # transpose masks (batches of 8 per PSUM bank)
                    maskT = mskTp.tile([P, KT, P], bf16, name="maskT")
                    for b0 in range(0, KT, 8):
                        nb = min(8, KT - b0)
                        psT = psTp.tile([P, 8, P], bf16, name="psTt")
                        for j in range(nb):
                            nc.tensor.transpose(psT[:, j, :],
                                                mask0[:, b0 + j, :],
                                                ident_sb[:])
                        nc.scalar.activation(out=maskT[:, b0:b0 + nb, :],
                                             in_=psT[:, 0:nb, :],
                                             func=Act.Copy)
                    # per-edge a_d via KT tiny matmuls
                    adpe = psC.tile([P, KT * HEADS], f32, name="adpe")
                    for k in range(KT):
                        nc.tensor.matmul(adpe[:, k * HEADS:(k + 1) * HEADS],
                                         lhsT=maskT[:, k, :],
                                         rhs=adS[:, t, :],
                                         start=True, stop=True)
                    # alpha = a_s + a_d ; leaky relu ; exp
                    alpha = smallp.tile([P, KT + 1, HEADS], f32, name="alpha")
                    nc.vector.tensor_tensor(
                        out=alpha[:, 0:KT, :], in0=g[:, 0:KT, NH:NH + HEADS],
                        in1=adpe[:].rearrange("p (k h) -> p k h", h=HEADS),
                        op=AluOp.add)
                    nc.vector.tensor_tensor(
                        out=alpha[:, KT, :], in0=g[:, KT, NH:NH + HEADS],
                        in1=adS[:, t, :], op=AluOp.add)
                    t2 = smallp.tile([P, KT + 1, HEADS], f32, name="t2")
                    nc.scalar.activation(out=t2[:], in_=alpha[:],
                                         func=Act.Copy, scale=0.2)
                    nc.vector.tensor_tensor(out=alpha[:], in0=alpha[:],
                                            in1=t2[:], op=AluOp.max)
                    ex = smallp.tile([P, KT + 1, HEADS], bf16, name="ex")
                    nc.scalar.activation(out=ex[:], in_=alpha[:],
                                         func=Act.Exp)
                    # scale all messages in place: g[:, :, h*HID:...] *= ex_h
                    gv = g[:, :, 0:NH].rearrange("p k (c h) -> p k c h",
                                                 h=HEADS)
                    nc.vector.tensor_tensor(
                        out=gv, in0=gv,
                        in1=ex[:, :, None, :].broadcast_to(
                            [P, KT + 1, HID, HEADS]),
                        op=AluOp.mult)
                    # scatter: out += mask.T @ msg ; den += mask.T @ ex
                    out_ps = psB.tile([P, NH], f32, name="outps")
                    den_ps = psB.tile([P, HEADS], f32, name="denps")
                    for k in range(KT + 1):
                        lhs = ident_sb[:] if k == KT else mask0[:, k, :]
                        first, last = (k == 0), (k == KT)
                        nc.tensor.matmul(out_ps[:, :], lhsT=lhs,
                                         rhs=g[:, k, 0:NH],
                                         start=first, stop=last)
                        nc.tensor.matmul(den_ps[:, :], lhsT=lhs,
                                         rhs=ex[:, k, :],
                                         start=first, stop=last)
                    # h = sum_h out_h / (4*den_h) + b
                    den4 = smallp.tile([P, HEADS], f32, name="den4")
                    nc.vector.tensor_scalar(
                        out=den4[:], in0=den_ps[:], scalar1=float(HEADS),
                        scalar2=float(HEADS) * 1e-16, op0=AluOp.mult,
                        op1=AluOp.add)
                    rec = smallp.tile([P, HEADS], f32, name="rec")
                    nc.vector.reciprocal(rec[:], den4[:])
                    tmp = accp.tile([P, HID, HEADS], f32, name="tmpacc")
                    opv = out_ps[:].rearrange("p (c h) -> p c h", h=HEADS)
                    for h in range(HEADS):
                        nc.scalar.activation(out=tmp[:, :, h],
                                             in_=opv[:, :, h], func=Act.Copy,
                                             scale=rec[:, h:h + 1])
                    acc = accp.tile([P, HID], f32, name="acc")
                    nc.vector.tensor_tensor(out=tmp[:, :, 0:2],
                                            in0=tmp[:, :, 0:2],
                                            in1=tmp[:, :, 2:4], op=AluOp.add)
                    nc.vector.tensor_tensor(out=tmp[:, :, 0],
                                            in0=tmp[:, :, 0],
                                            in1=tmp[:, :, 1], op=AluOp.add)
                    nc.vector.tensor_tensor(out=acc[:], in0=tmp[:, :, 0],
                                            in1=bbc_sb[:], op=AluOp.add)
                    writer(t, acc, psTp)

            def write_h(t, acc, psTp):
                accb = accp.tile([P, HID], bf16, name="accb")
                nc.scalar.activation(out=accb[:], in_=acc[:], func=Act.Copy)
                tp = psTp.tile([P, 8, P], bf16, name="psTt")
                nc.tensor.transpose(tp[:, 0, :], accb[:], ident_sb[:])
                nc.scalar.activation(out=hT_sb[:, t * P:(t + 1) * P],
                                     in_=tp[:, 0, :], func=Act.Copy)

            def write_out(t, acc, psTp):
                nc.sync.dma_start(out=t_out[t * P:(t + 1) * P, :], in_=acc[:])

            def ag(Go, Gf):
                nc.gpsimd.collective_compute(
                    "AllGather", AluOp.bypass,
                    replica_groups=[list(range(NCORES))],
                    ins=[Go[:, :].rearrange("r w -> (r w)").unsqueeze(0)
                         .opt()],
                    outs=[Gf[:, :].rearrange("(o r) w -> o (r w)",
                                             o=NCORES).opt()])

            def ag_chunk(Go_, Gf, row0, row1):
                nc.gpsimd.collective_compute(
                    "AllGather", AluOp.bypass,
                    replica_groups=[list(range(NCORES))],
                    ins=[Go_[row0:row1, :]
                         .rearrange("r w -> (r w)").unsqueeze(0).opt()],
                    outs=[Gf[:, :].rearrange("(o r) w -> o (r w)", o=NCORES)
                          [:, row0 * GW2:row1 * GW2].opt()])

            G2ov = G2o[:, :].rearrange("(b p q) w -> q b p w", q=2, p=P)
            psA2holder = []

            # layer-2 phase A for one tile (h tile just written to hT_sb)
            def phA2_tile(t):
                psA2 = psA2holder[0]
                stg = stagep.tile([P, GW], bf16, name="stg2")
                ps = psA2.tile([P, NH], f32, name="a2ps")
                nc.tensor.matmul(
                    ps[:, :], lhsT=hT_sb[:, t * P:(t + 1) * P],
                    rhs=w2_sb[:, 0:NH], start=True, stop=True)
                nc.scalar.activation(out=stg[:, 0:NH], in_=ps[:, :],
                                     func=Act.Copy)
                ps2 = psA2.tile([P, NH], f32, name="a2ps")
                nc.tensor.matmul(
                    ps2[:, 0:GW - NH], lhsT=hT_sb[:, t * P:(t + 1) * P],
                    rhs=w2_sb[:, NH:GW], start=True, stop=True)
                nc.scalar.activation(out=stg[:, NH:GW],
                                     in_=ps2[:, 0:GW - NH], func=Act.Copy)
                nc.sync.dma_start(out=G2ov[t & 1, t >> 1, :, 0:GW],
                                  in_=stg[:])

            AGCH = TPC          # single AG2 (Shared DRAM: one writer)

            def write_h2(t, acc, psTp):
                write_h(t, acc, psTp)
                phA2_tile(t)
                if (t + 1) % AGCH == 0 and (t & 1) == 1:
                    ag_chunk(G2o, G2, (t + 1 - AGCH) * P, (t + 1) * P)

            import os
            PH = int(os.environ.get("KPH", "5"))
            with nc.named_scope("phA1"):
                phase_a(G1o, w1_sb, x_sb)
            with nc.named_scope("phAG1"):
                ag(G1o, G1)
            if PH >= 2:
                with nc.named_scope("phB1"), \
                     tc.tile_pool(name="psA2", bufs=1, space="PSUM") as psA2p:
                    psA2holder.append(psA2p)
                    phase_b(G1, G1o, b1_sb,
                            write_h2 if PH >= 4 else write_h)
            if PH >= 5:
                with nc.named_scope("phB2"):
                    phase_b(G2, G2o, b2_sb, write_out)

    nc.compile()
    return nc


def _sched_gather_info(nc):
    """Final-order scheduled positions of InstDMAGatherAnt, name -> pos."""
    pos = {}
    i = 0
    for blk in nc.m.functions[0].blocks:
        for inst in blk.instructions:
            if type(inst).__name__ == "InstDMAGatherAnt":
                pos[inst.name] = i
                i += 1
    return pos


def _queue_consistent(nc):
    """Each DMA semaphore must be incremented by exactly one SWDGE queue."""
    sem_q = {}
    for blk in nc.m.functions[0].blocks:
        for inst in blk.instructions:
            if type(inst).__name__ != "InstDMAGatherAnt":
                continue
            si = inst.sync_info
            if not si or not si.on_update:
                continue
            sem = si.on_update[0].id
            q = inst.queue_num
            if sem_q.setdefault(sem, q) != q:
                return False
    return True


def _build_multiq(s):
    nc = _build(s, None)
    names = [g.ins.name for g in nc._gat_insts]
    for _ in range(3):
        pos = _sched_gather_info(nc)
        qmap = [pos[n] % 4 for n in names]
        nc2 = _build(s, qmap)
        if _queue_consistent(nc2):
            return nc2
        nc = nc2
        names = [g.ins.name for g in nc._gat_insts]
    return _build(s, None)                               # safe fallback


def _get_nc(s):
    key = tuple(sorted(s.items()))
    if key not in _CACHE:
        _CACHE[key] = _build_multiq(s)
    return _CACHE[key]


def _in_maps(shared, percore):
    maps = []
    for i in range(NCORES):
        maps.append({"xT": percore[i]["xT"], "W1cat": shared["W1cat"],
                     "W2cat": shared["W2cat"], "IOTA": shared["IOTA"],
                     "IDENT": shared["IDENT"], "B1bc": shared["B1bc"],
                     "B2bc": shared["B2bc"], "IXL": percore[i]["IXL"],
                     "IXH": percore[i]["IXH"], "DSTL": percore[i]["DSTL"],
                     "OWNL": percore[i]["OWNL"],
                     "OWNH": percore[i]["OWNH"]})
    return maps


def kernel(**inputs):
    from concourse import bass_utils

    x = np.asarray(inputs["x"], dtype=np.float32)
    edge_index = np.asarray(inputs["edge_index"])
    args = (x, edge_index,
            np.asarray(inputs["W1"], np.float32),
            np.asarray(inputs["att_src1"], np.float32),
            np.asarray(inputs["att_dst1"], np.float32),
            np.asarray(inputs["b1"], np.float32),
            np.asarray(inputs["W2"], np.float32),
            np.asarray(inputs["att_src2"], np.float32),
            np.asarray(inputs["att_dst2"], np.float32),
            np.asarray(inputs["b2"], np.float32))
    shapes, shared, percore = _host_prep(*args)
    nc = _get_nc(shapes)
    res = bass_utils.run_bass_kernel_spmd(nc, _in_maps(shared, percore),
                                          core_ids=list(range(NCORES)))
    out_cat = np.concatenate(
        [res.results[i]["out"] for i in range(NCORES)], axis=0)
    out = out_cat[shared["newid"][:shapes["N"]]]
    return np.ascontiguousarray(out, dtype=np.float32)
